# revision 10
# baseline (speedup 1.0000x reference)
"""DeepSets segment-reduce kernel for 8 Trainium2 NeuronCores.

Math: out[s] = sum_{i in s} (x_i @ W + b) = (sum_{i in s} x_i) @ W + count_s * b.
So the device only needs per-segment sums of the 2-dim points plus counts;
the [N, 64] intermediate never exists.

Sharding (per the contiguous-set-range hint): host splits the sorted
segment_ids at segment boundaries - core k owns segments [512k, 512k+512)
and their contiguous point range. Boundary offsets (index metadata) are
found with searchsorted on the host; all arithmetic on x happens on device.

Device algorithm per core (512 segments = 4 groups x 128 partitions):
  1. indirect DMA gather: slot (p, g) <- LMAX points starting at segment
     (g*128+p)'s first point (fixed-length, tail overlaps next segments).
  2. DVE: masked = (iota < 2*len) * slot; strided reduce -> per-segment
     sums of x0, x1. Counts (host metadata) fill a third column.
  3. PE: transpose [128, 3] -> [3, 128] per group; 4 matmuls
     out[128seg, 64] = S3[3, 128seg].T @ W3[3, 64], W3 = [W; b].
  4. DVE copy PSUM->SBUF, DMA -> DRAM output rows.

DEEPSETS_BENCH_ITERS=k repeats the on-device body k times (same data, same
result) so test.py can wall-clock (k2-k1) deltas for HW exec time.
"""

import os
from contextlib import ExitStack

import numpy as np

import concourse.bass as bass
import concourse.mybir as mybir
from concourse.bass_utils import run_bass_kernel_spmd

P = 128
G = 4                 # segment groups per core
CORES = 8
NUM_SEGMENTS = 4096
SEGC = NUM_SEGMENTS // CORES     # 512 segments per core
FEAT = 64

_kernel_cache: dict = {}

# per-iteration semaphore increments
_META_INC = 9 * 16    # 5 meta-input DMAs + 4 output DMAs
_DVE_INC = 5          # sums12-ready + 4 s3t copies
_TP_INC = G
_MM_INC = G
_OUTC_INC = G


def _build(LMAX: int, NPAD: int, iters: int) -> bass.Bass:
    SLOT = 2 * LMAX   # f32 elements per gathered slot
    f32 = mybir.dt.float32
    nc = bass.Bass()

    xs = nc.dram_tensor("xs", [NPAD, 2], f32, kind="ExternalInput")
    offs = nc.dram_tensor("offs", [P, G], mybir.dt.int32, kind="ExternalInput")
    thr = nc.dram_tensor("thr", [P, G], f32, kind="ExternalInput")
    cnt = nc.dram_tensor("cnt", [P, G], f32, kind="ExternalInput")
    w3 = nc.dram_tensor("w3", [3, FEAT], f32, kind="ExternalInput")
    ident = nc.dram_tensor("ident", [P, P], f32, kind="ExternalInput")
    out = nc.dram_tensor("out", [SEGC, FEAT], f32, kind="ExternalOutput")

    with ExitStack() as ctx:
        offs_t = ctx.enter_context(nc.sbuf_tensor("offs_t", [P, G], mybir.dt.int32))
        thr_t = ctx.enter_context(nc.sbuf_tensor("thr_t", [P, G], f32))
        cnt_t = ctx.enter_context(nc.sbuf_tensor("cnt_t", [P, G], f32))
        w3_t = ctx.enter_context(nc.sbuf_tensor("w3_t", [3, FEAT], f32))
        ident_t = ctx.enter_context(nc.sbuf_tensor("ident_t", [P, P], f32))
        iota_t = ctx.enter_context(nc.sbuf_tensor("iota_t", [P, SLOT], f32))
        gx = [
            ctx.enter_context(nc.sbuf_tensor(f"gx{g}", [P, SLOT], f32))
            for g in range(G)
        ]
        masked = ctx.enter_context(nc.sbuf_tensor("masked", [P, SLOT], f32))
        outb = ctx.enter_context(nc.sbuf_tensor("outb", [P, G * FEAT], f32))
        sums12 = ctx.enter_context(nc.sbuf_tensor("sums12", [P, 3 * G], f32))
        s3tg = [
            ctx.enter_context(nc.sbuf_tensor(f"s3t{g}", [3, P], f32))
            for g in range(G)
        ]
        psT = [
            ctx.enter_context(nc.psum_tensor(f"psT{g}", [3, P], f32))
            for g in range(G)
        ]
        po = [
            ctx.enter_context(nc.psum_tensor(f"po{g}", [P, FEAT], f32))
            for g in range(G)
        ]
        meta_sem = ctx.enter_context(nc.semaphore("meta"))
        gsem = [ctx.enter_context(nc.semaphore(f"g{g}")) for g in range(G)]
        dve_sem = ctx.enter_context(nc.semaphore("dve"))
        tp_sem = ctx.enter_context(nc.semaphore("tp"))
        mm_sem = ctx.enter_context(nc.semaphore("mm"))
        outc_sem = ctx.enter_context(nc.semaphore("outc"))
        block = ctx.enter_context(nc.Block())
        NMETA = 5  # offs, thr, cnt, w3, ident

        @block.sync
        def _(sync):
            for it in range(iters):
                mb = it * _META_INC
                sync.dma_start(offs_t[:, :], offs[:, :]).then_inc(meta_sem, 16)
                sync.dma_start(thr_t[:, :], thr[:, :]).then_inc(meta_sem, 16)
                sync.dma_start(cnt_t[:, :], cnt[:, :]).then_inc(meta_sem, 16)
                sync.dma_start(w3_t[:, :], w3[:, :]).then_inc(meta_sem, 16)
                sync.dma_start(ident_t[:, :], ident[:, :]).then_inc(meta_sem, 16)
                for g in range(G):
                    sync.wait_ge(outc_sem, it * _OUTC_INC + g + 1)
                    sync.dma_start(
                        out[g * P:(g + 1) * P, :], outb[:, g * FEAT:(g + 1) * FEAT]
                    ).then_inc(meta_sem, 16)

        @block.gpsimd
        def _(gpsimd):
            gpsimd.iota(
                iota_t[:, :],
                pattern=[[1, SLOT]],
                base=0,
                channel_multiplier=0,
                allow_small_or_imprecise_dtypes=True,
            )
            for it in range(iters):
                # waiting for this iter's meta DMAs also guarantees the
                # previous iter's output DMAs (same sem) -> gx reuse is safe
                gpsimd.wait_ge(meta_sem, it * _META_INC + 16 * NMETA)
                for g in range(G):
                    gpsimd.indirect_dma_start(
                        out=gx[g][:, :],
                        out_offset=None,
                        in_=xs[:, :],
                        in_offset=bass.IndirectOffsetOnAxis(
                            ap=offs_t[:, g:g + 1], axis=0
                        ),
                    ).then_inc(gsem[g], 16)

        @block.vector
        def _(vector):
            for it in range(iters):
                vector.wait_ge(meta_sem, it * _META_INC + 16 * NMETA)
                if it > 0:
                    # don't overwrite sums12 while PE transposes still read it
                    vector.wait_ge(tp_sem, it * _TP_INC)
                for g in range(G):
                    vector.wait_ge(gsem[g], (it + 1) * 16)
                    # masked = (iota < 2*len) * gathered
                    nc.vector.scalar_tensor_tensor(
                        out=masked[:, :],
                        in0=iota_t[:, :],
                        scalar=thr_t[:, g:g + 1],
                        in1=gx[g][:, :],
                        op0=mybir.AluOpType.is_lt,
                        op1=mybir.AluOpType.mult,
                    )
                    # per-component sums: view [P, (i c)] -> [P, c, i], reduce i
                    nc.vector.reduce_sum(
                        out=sums12[:, 3 * g:3 * g + 2],
                        in_=masked[:, :].rearrange("p (i c) -> p c i", c=2),
                        axis=mybir.AxisListType.X,
                    )
                # counts into third columns
                for g in range(G):
                    nc.vector.tensor_copy(
                        out=sums12[:, 3 * g + 2:3 * g + 3], in_=cnt_t[:, g:g + 1]
                    )
                nc.vector.tensor_copy(
                    out=sums12[:, 0:1], in_=sums12[:, 0:1]
                ).then_inc(dve_sem, 1)
                for g in range(G):
                    vector.wait_ge(tp_sem, it * _TP_INC + g + 1)
                    nc.vector.tensor_copy(
                        out=s3tg[g][:, :], in_=psT[g][:, :]
                    ).then_inc(dve_sem, 1)
                for g in range(G):
                    vector.wait_ge(mm_sem, it * _MM_INC + g + 1)
                    if it > 0:
                        # previous iter's output DMA g must be done with outb
                        vector.wait_ge(
                            meta_sem, (it - 1) * _META_INC + 16 * NMETA + 16 * (g + 1)
                        )
                    nc.vector.tensor_copy(
                        out=outb[:, g * FEAT:(g + 1) * FEAT], in_=po[g][:, :]
                    ).then_inc(outc_sem, 1)

        @block.tensor
        def _(tensor):
            for it in range(iters):
                tensor.wait_ge(dve_sem, it * _DVE_INC + 1)
                for g in range(G):
                    nc.tensor.transpose(
                        out=psT[g][:, :],
                        in_=sums12[:, 3 * g:3 * g + 3],
                        identity=ident_t[:, :],
                    ).then_inc(tp_sem, 1)
                for g in range(G):
                    tensor.wait_ge(dve_sem, it * _DVE_INC + 2 + g)
                    nc.tensor.matmul(
                        out=po[g][:, :],
                        lhsT=s3tg[g][:, :],
                        rhs=w3_t[:, :],
                        start=True,
                        stop=True,
                    ).then_inc(mm_sem, 1)

    return nc


def _get_kernel(LMAX: int, NPAD: int, iters: int) -> bass.Bass:
    key = (LMAX, NPAD, iters)
    if key not in _kernel_cache:
        _kernel_cache[key] = _build(LMAX, NPAD, iters)
    return _kernel_cache[key]


def kernel(x, segment_ids, W, b, num_segments, **_unused):
    x = np.ascontiguousarray(np.asarray(x, dtype=np.float32))
    ids = np.asarray(segment_ids)
    W = np.asarray(W, dtype=np.float32)
    b = np.asarray(b, dtype=np.float32)
    S = int(num_segments)
    assert S == NUM_SEGMENTS, f"kernel hardcoded for {NUM_SEGMENTS} segments, got {S}"
    N = x.shape[0]
    assert ids.shape == (N,)
    iters = int(os.environ.get("DEEPSETS_BENCH_ITERS", "1"))

    # host index metadata: segment boundaries in the sorted id array
    bounds = np.searchsorted(ids, np.arange(S + 1), side="left").astype(np.int64)
    lens = np.diff(bounds)
    lmax_data = int(lens.max()) if N else 1
    LMAX = max(64, ((lmax_data + 63) // 64) * 64)

    core_starts = bounds[0:S:SEGC]            # first point of each core
    core_ends = bounds[SEGC:S + 1:SEGC]
    core_pts = core_ends - core_starts
    NPAD = int(((core_pts.max() + LMAX + 127) // 128) * 128)

    nc = _get_kernel(LMAX, NPAD, iters)

    w3 = np.concatenate([W, b[None, :]], axis=0).astype(np.float32)  # [3, 64]
    ident = np.eye(P, dtype=np.float32)

    in_maps = []
    for c in range(CORES):
        p0, p1 = int(core_starts[c]), int(core_ends[c])
        xs = np.zeros((NPAD, 2), np.float32)
        xs[: p1 - p0] = x[p0:p1]
        seg0 = c * SEGC
        rel = (bounds[seg0:seg0 + SEGC] - p0).astype(np.int32)   # [512]
        ln = lens[seg0:seg0 + SEGC].astype(np.float32)           # [512]
        # slot (p, g) = segment g*128 + p
        offs = rel.reshape(G, P).T.copy()                        # [128, 4] int32
        thr = (2.0 * ln).reshape(G, P).T.copy()                  # [128, 4] f32
        cntm = ln.reshape(G, P).T.copy()                         # [128, 4] f32
        in_maps.append(
            {
                "xs": xs,
                "offs": offs,
                "thr": thr,
                "cnt": cntm,
                "w3": w3,
                "ident": ident,
            }
        )

    res = run_bass_kernel_spmd(nc, in_maps, core_ids=list(range(CORES)))
    out = np.concatenate([res.results[c]["out"] for c in range(CORES)], axis=0)
    return out.astype(np.float32)


# revision 14
# speedup vs baseline: 4.2791x; 4.2791x over previous
"""DeepSets segment-reduce kernel for 8 Trainium2 NeuronCores.

Math: out[s] = sum_{i in s} (x_i @ W + b) = (sum_{i in s} x_i) @ W + count_s * b.
The device only needs per-segment sums of the 2-dim points plus counts; the
[N, 64] intermediate never exists.

Sharding (contiguous-set-range hint): host splits the sorted segment_ids at
segment boundaries - core k owns segments [512k, 512k+512) and their
contiguous point range. Boundary offsets are host index metadata
(searchsorted); all arithmetic on x runs on device.

Device layout per core: 512 segments = 4 groups x 128 partitions; slot
(p, g) holds segment g*128+p. Segments are near-uniform (~977 pts), so slot
starts are nearly affine in the slot index: ONE 3-d strided DMA
(stride D = mean spacing, chunk-base fixups folded into a host-side chunk
relayout of the slab) over-gathers a fixed window of L points per slot.
The true segment [h, h+len) inside each window is selected with an
unsigned-wrap mask: (iota - 2h) u< 2len, then one strided reduce produces
per-segment component sums. Counts ride in with the metadata blob. One PE
transpose + one block-diagonal matmul (W12[12, 256]) turn [128, 12] sums
into all 512 output rows at once.

Instruction count per core per iteration: 3 DMA + 4 big DVE ops + 2 DVE
copies + 2 PE ops (this environment charges ~10-30us per instruction, so
instruction count dominates).

DEEPSETS_BENCH_ITERS=k repeats the body k times for wall-clock delta timing.
"""

import os
from contextlib import ExitStack

import numpy as np

import concourse.bass as bass
import concourse.mybir as mybir
from concourse.bass_utils import run_bass_kernel_spmd

P = 128
G = 4
CORES = 8
NUM_SEGMENTS = 4096
SEGC = NUM_SEGMENTS // CORES     # 512
FEAT = 64
BLOB_W = 12 + G * FEAT + P       # meta cols + W12 + identity = 396

_kernel_cache: dict = {}

_DSEM_INC = 3 * 16   # blob + gather + out DMAs per iter
_DVE_INC = 3         # reduce, s3t copy, outb copy
_PE_INC = 2          # transpose, matmul


def _build(D: int, L: int, CS: int, iters: int) -> bass.Bass:
    """D: slot stride (points); L: window length (points); CS: f32 elems per
    chunk slab (>= 2*(127*D + L))."""
    SLOT2 = 2 * L
    f32 = mybir.dt.float32
    i32 = mybir.dt.int32
    u32 = mybir.dt.uint32
    nc = bass.Bass()

    xsB = nc.dram_tensor("xsB", [G, CS], f32, kind="ExternalInput")
    blob = nc.dram_tensor("blob", [P, BLOB_W], f32, kind="ExternalInput")
    outd = nc.dram_tensor("outd", [P, G * FEAT], f32, kind="ExternalOutput")

    with ExitStack() as ctx:
        meta_t = ctx.enter_context(nc.sbuf_tensor("meta_t", [P, BLOB_W], f32))
        iota_t = ctx.enter_context(nc.sbuf_tensor("iota_t", [P, SLOT2], i32))
        gx = ctx.enter_context(nc.sbuf_tensor("gx", [P, G * SLOT2], f32))
        tmp = ctx.enter_context(nc.sbuf_tensor("tmp", [P, G * SLOT2], i32))
        s3t = ctx.enter_context(nc.sbuf_tensor("s3t", [12, P], f32))
        outb = ctx.enter_context(nc.sbuf_tensor("outb", [P, G * FEAT], f32))
        psum12 = ctx.enter_context(nc.psum_tensor("psum12", [12, P], f32))
        pso = ctx.enter_context(nc.psum_tensor("pso", [P, G * FEAT], f32))
        dsem = ctx.enter_context(nc.semaphore("dsem"))
        iot_sem = ctx.enter_context(nc.semaphore("iot"))
        dve_sem = ctx.enter_context(nc.semaphore("dve"))
        pe_sem = ctx.enter_context(nc.semaphore("pe"))
        block = ctx.enter_context(nc.Block())

        # device-side views
        meta_u = meta_t[:, :].bitcast(u32)
        h2_b = bass.AP(
            tensor=meta_u.tensor, offset=0,
            ap=[[BLOB_W, P], [1, G], [0, SLOT2]],
        )
        len2_b = bass.AP(
            tensor=meta_u.tensor, offset=4,
            ap=[[BLOB_W, P], [1, G], [0, SLOT2]],
        )
        iota_b = bass.AP(
            tensor=iota_t[:, :].bitcast(u32).tensor, offset=0,
            ap=[[SLOT2, P], [0, G], [1, SLOT2]],
        )
        tmp_u = tmp[:, :].bitcast(u32)
        tmp_f = tmp[:, :].bitcast(f32)
        # gather source: [p(slot), g(chunk), f] from xsB
        gather_src = bass.AP(
            tensor=xsB[:, :].tensor, offset=0,
            ap=[[2 * D, P], [CS, G], [1, SLOT2]],
        )
        w12_ap = meta_t[0:12, 12:12 + G * FEAT]
        ident_ap = meta_t[:, 12 + G * FEAT:BLOB_W]
        sums_out = meta_t[:, 0:8].rearrange("p (g c) -> p g c", c=2)
        gx_red = bass.AP(
            tensor=gx[:, :].tensor, offset=0,
            ap=[[G * SLOT2, P], [SLOT2, G], [1, 2], [2, L]],
        )

        @block.sync
        def _(sync):
            for it in range(iters):
                db = it * _DSEM_INC
                sync.dma_start(meta_t[:, :], blob[:, :]).then_inc(dsem, 16)
                sync.dma_start(
                    bass.AP(tensor=gx[:, :].tensor, offset=0,
                            ap=[[G * SLOT2, P], [SLOT2, G], [1, SLOT2]]),
                    gather_src,
                ).then_inc(dsem, 16)
                sync.wait_ge(dve_sem, it * _DVE_INC + 3)
                sync.dma_start(outd[:, :], outb[:, :]).then_inc(dsem, 16)

        @block.gpsimd
        def _(gpsimd):
            gpsimd.iota(
                iota_t[:, :], pattern=[[1, SLOT2]], base=0,
                channel_multiplier=0,
            ).then_inc(iot_sem, 1)

        @block.vector
        def _(vector):
            vector.wait_ge(iot_sem, 1)
            for it in range(iters):
                vector.wait_ge(dsem, it * _DSEM_INC + 32)
                # tmp = iota - 2h  (u32, wraps negative -> huge)
                nc.vector.tensor_tensor(
                    out=tmp_u, in0=iota_b, in1=h2_b,
                    op=mybir.AluOpType.subtract,
                )
                # tmp = (tmp u< 2len) as f32 mask (in-place)
                nc.vector.tensor_tensor(
                    out=tmp_f, in0=tmp_u, in1=len2_b,
                    op=mybir.AluOpType.is_lt,
                )
                # gx = mask * gx (in-place on in1)
                nc.vector.tensor_tensor(
                    out=gx[:, :], in0=tmp_f, in1=gx[:, :],
                    op=mybir.AluOpType.mult,
                )
                # per-(group, comp) sums -> meta cols 0..7
                nc.vector.reduce_sum(
                    out=sums_out, in_=gx_red, axis=mybir.AxisListType.X,
                ).then_inc(dve_sem, 1)
                vector.wait_ge(pe_sem, it * _PE_INC + 1)
                nc.vector.tensor_copy(out=s3t[:, :], in_=psum12[:, :]).then_inc(
                    dve_sem, 1
                )
                vector.wait_ge(pe_sem, it * _PE_INC + 2)
                nc.vector.tensor_copy(out=outb[:, :], in_=pso[:, :]).then_inc(
                    dve_sem, 1
                )

        @block.tensor
        def _(tensor):
            for it in range(iters):
                tensor.wait_ge(dve_sem, it * _DVE_INC + 1)
                nc.tensor.transpose(
                    out=psum12[:, :], in_=meta_t[:, 0:12], identity=ident_ap,
                ).then_inc(pe_sem, 1)
                tensor.wait_ge(dve_sem, it * _DVE_INC + 2)
                nc.tensor.matmul(
                    out=pso[:, :], lhsT=s3t[:, :], rhs=w12_ap,
                    start=True, stop=True,
                ).then_inc(pe_sem, 1)

    return nc


def _get_kernel(D: int, L: int, CS: int, iters: int) -> bass.Bass:
    key = (D, L, CS, iters)
    if key not in _kernel_cache:
        _kernel_cache[key] = _build(D, L, CS, iters)
    return _kernel_cache[key]


def _plan(bounds: np.ndarray, lens: np.ndarray):
    """Global stride D, window L, per-(core,chunk) bases and per-slot h."""
    D = max(1, int(round(bounds[-1] / NUM_SEGMENTS)))
    bases = np.zeros((CORES, G), np.int64)
    hs = np.zeros((CORES, G, P), np.int64)
    L = 1
    j = np.arange(P)
    for c in range(CORES):
        st = bounds[c * SEGC:(c + 1) * SEGC] - bounds[c * SEGC]
        for g in range(G):
            sj = st[g * P:(g + 1) * P]
            lj = lens[c * SEGC + g * P:c * SEGC + (g + 1) * P]
            base = int((sj - j * D).min())
            h = sj - (base + j * D)
            bases[c, g] = base
            hs[c, g] = h
            L = max(L, int((h + lj).max()))
    L = ((L + 63) // 64) * 64
    return D, L, bases, hs


def kernel(x, segment_ids, W, b, num_segments, **_unused):
    x = np.ascontiguousarray(np.asarray(x, dtype=np.float32))
    ids = np.asarray(segment_ids)
    W = np.asarray(W, dtype=np.float32)
    b = np.asarray(b, dtype=np.float32)
    S = int(num_segments)
    assert S == NUM_SEGMENTS, f"kernel hardcoded for {NUM_SEGMENTS} segments"
    N = x.shape[0]
    iters = int(os.environ.get("DEEPSETS_BENCH_ITERS", "1"))

    bounds = np.searchsorted(ids, np.arange(S + 1), side="left").astype(np.int64)
    lens = np.diff(bounds)
    D, L, bases, hs = _plan(bounds, lens)
    SLOT2 = 2 * L
    CS = ((2 * ((P - 1) * D + L) + 127) // 128) * 128

    nc = _get_kernel(D, L, CS, iters)

    # W12 block-diagonal [12, 256]: rows 2g+c -> W[c], rows 8+g -> b
    w12 = np.zeros((12, G * FEAT), np.float32)
    for g in range(G):
        for c2 in range(2):
            w12[2 * g + c2, g * FEAT:(g + 1) * FEAT] = W[c2]
        w12[8 + g, g * FEAT:(g + 1) * FEAT] = b
    ident = np.eye(P, dtype=np.float32)

    xflat = x.reshape(-1)
    in_maps = []
    for c in range(CORES):
        p0, p1 = int(bounds[c * SEGC]), int(bounds[(c + 1) * SEGC])
        xsB = np.zeros((G, CS), np.float32)
        for g in range(G):
            a0 = 2 * (p0 + int(bases[c, g]))
            a1 = a0 + CS
            lo, hi = max(a0, 0), min(a1, 2 * N)
            if hi > lo:
                xsB[g, lo - a0:hi - a0] = xflat[lo:hi]
        blobv = np.zeros((P, BLOB_W), np.float32)
        seg0 = c * SEGC
        h2 = (2 * hs[c].astype(np.int64)).astype(np.int32)          # [G, P]
        ln2 = (2 * lens[seg0:seg0 + SEGC].reshape(G, P)).astype(np.int32)
        blobv[:, 0:G] = h2.T.view(np.float32) if h2.T.flags.c_contiguous else \
            np.ascontiguousarray(h2.T).view(np.float32)
        blobv[:, G:2 * G] = np.ascontiguousarray(ln2.T).view(np.float32)
        blobv[:, 2 * G:3 * G] = lens[seg0:seg0 + SEGC].reshape(G, P).T
        blobv[0:12, 12:12 + G * FEAT] = w12
        blobv[:, 12 + G * FEAT:BLOB_W] = ident
        in_maps.append({"xsB": xsB, "blob": blobv})

    res = run_bass_kernel_spmd(nc, in_maps, core_ids=list(range(CORES)))
    parts = [
        res.results[c]["outd"].reshape(P, G, FEAT).transpose(1, 0, 2).reshape(
            SEGC, FEAT
        )
        for c in range(CORES)
    ]
    return np.concatenate(parts, axis=0).astype(np.float32)
